# revision 10
# baseline (speedup 1.0000x reference)
"""Decode attention (q_len=1) Bass kernel for Trainium2, sharded over heads on 8 cores.

Problem: q [8,32,1,128], k/v [8,32,4096,128], mask [8,1,1,4096] (f32).
Each core handles 4 heads -> 32 (batch, head) pairs; per pair it streams one
merged K/V slab from HBM (memory-bound).

Layout trick: K and V ride the PE *weight* port as self-loading matmuls with an
N=1 moving operand, producing scores^T [s-on-partitions] so the softmax (exp
via ACT with fused scale + accum_out row-sums) is lane-parallel and no on-chip
transposes are needed. Output is returned as out^T [128, 32] plus softmax
denominators [32]; the host does the final divide/transpose.

q is always carried as an fp16 hi/lo pair (host-split) and probs are split
hi/lo on-chip, so neither contributes rounding error beyond ~2^-22. The
variants differ only in how k/v slabs are encoded (DMA bytes vs accuracy):

  f16   - k, v single fp16 slab each (2B/elem DMA): fastest, err ~3e-4
  f16x2 - k, v fp16 hi+lo slabs (4B/elem DMA): fp32-grade err ~3.5e-6
  f32   - plain fp32 matmuls (4x PE row cost; PE-bound, reference only)
"""

import sys

sys.path.insert(0, "/opt/trn_rl_repo")

import numpy as np

import concourse.bass as bass
import concourse.bacc as bacc
import concourse.mybir as mybir
import concourse.tile as tile
from concourse.bass_utils import run_bass_kernel_spmd

B = 8
H = 32
D = 128
S = 4096
NCORES = 8
HL = H // NCORES          # heads per core
PAIRS = B * HL            # (batch, head) pairs per core
C = S // 128              # 128-row chunks along sequence
SCALE = float(D) ** -0.5

MM_VARIANT = "f16x2"

_PROGRAMS = {}


def _cfg(variant):
    f16 = mybir.dt.float16
    f32 = mybir.dt.float32
    if variant == "f16":
        # kv slab = [k, v]; scores: k@(qh, ql); V: v@(ph, pl)
        return dict(dt=f16, nk=1, nv=1,
                    smm=[(0, 0), (0, 1)], vmm=[(0, 0), (0, 1)])
    if variant == "f16x2":
        # kv slab = [kh, kl, vh, vl]
        return dict(dt=f16, nk=2, nv=2,
                    smm=[(0, 0), (1, 0), (0, 1)], vmm=[(0, 0), (1, 0), (0, 1)])
    if variant == "f32":
        return dict(dt=f32, nk=1, nv=1, smm=[(0, 0)], vmm=[(0, 0)])
    raise ValueError(variant)


def _build_program(variant):
    f32 = mybir.dt.float32
    cfg = _cfg(variant)
    mdt = cfg["dt"]
    nk, nv = cfg["nk"], cfg["nv"]
    nsl = nk + nv
    nq = 2 if mdt is not f32 else 1

    nc = bacc.Bacc("TRN2", target_bir_lowering=False, debug=False, num_devices=NCORES)

    qT_d = nc.dram_tensor("qT", [D, nq, PAIRS], mdt, kind="ExternalInput").ap()
    kv_d = nc.dram_tensor("kv", [PAIRS, D, nsl, S], mdt, kind="ExternalInput").ap()
    maskT_d = nc.dram_tensor("maskT", [D, B * C], f32, kind="ExternalInput").ap()
    outT_d = nc.dram_tensor("outT", [D, PAIRS], f32, kind="ExternalOutput").ap()
    den_d = nc.dram_tensor("den", [PAIRS, 1], f32, kind="ExternalOutput").ap()

    with tile.TileContext(nc) as tc:
        with (
            tc.tile_pool(name="kvslab", bufs=5) as kvpool,
            tc.tile_pool(name="probs", bufs=2) as ppool,
            tc.tile_pool(name="small", bufs=1) as small,
            tc.tile_pool(name="psc", bufs=2, space=bass.MemorySpace.PSUM) as psc_pool,
            tc.tile_pool(name="pout", bufs=2, space=bass.MemorySpace.PSUM) as pout_pool,
            tc.tile_pool(name="pden", bufs=1, space=bass.MemorySpace.PSUM) as pden_pool,
        ):
            qT = small.tile([D, nq, PAIRS], mdt)
            nc.sync.dma_start(qT[:], qT_d[:])
            maskT = small.tile([D, B * C], f32)
            nc.sync.dma_start(maskT[:], maskT_d[:])
            ones = small.tile([D, 1], f32)
            nc.vector.memset(ones[:], 1.0)
            partials = small.tile([D, PAIRS], f32)
            outT_sb = small.tile([D, PAIRS], f32)

            def emit_v_product(p, kv, pbs):
                # out^T_p = sum_c v_chunk^T @ probs^T_chunk  -> [128 d, 1]
                ot = pout_pool.tile([D, 1], f32, tag="pout")
                for c in range(C):
                    cs = slice(c * 128, (c + 1) * 128)
                    for i, (vi, pi) in enumerate(cfg["vmm"]):
                        nc.tensor.matmul(
                            ot[:, 0:1],
                            kv[:, nk + vi, cs],
                            pbs[pi][:, c : c + 1],
                            start=(c == 0 and i == 0),
                            stop=(c == C - 1 and i == len(cfg["vmm"]) - 1),
                        )
                nc.vector.tensor_copy(outT_sb[:, p : p + 1], ot[:, 0:1])

            # Software pipeline: V-product of pair p-1 is emitted AFTER the
            # scores of pair p, so the softmax chain (DVE/ACT) of pair p
            # overlaps PE work instead of stalling it.
            pending = None
            for p in range(PAIRS):
                b = p // HL
                kv = kvpool.tile([D, nsl, S], mdt, tag="kvslab")
                nc.sync.dma_start(kv[:], kv_d[p])

                # scores^T: column c = sum of k_slab @ q_col  -> [128 s, 1]
                sc = psc_pool.tile([128, C], f32, tag="psc")
                for c in range(C):
                    cs = slice(c * 128, (c + 1) * 128)
                    for i, (ki, qi) in enumerate(cfg["smm"]):
                        nc.tensor.matmul(
                            sc[:, c : c + 1],
                            kv[:, ki, cs],
                            qT[:, qi, p : p + 1],
                            start=(i == 0),
                            stop=(i == len(cfg["smm"]) - 1),
                        )
                # + mask/SCALE (host pre-divided), then exp(SCALE * x)
                nc.vector.tensor_add(sc[:], sc[:], maskT[:, b * C : (b + 1) * C])
                pb = ppool.tile([128, C], f32, tag="probs")
                nc.scalar.activation(
                    pb[:], sc[:], mybir.ActivationFunctionType.Exp,
                    scale=SCALE, accum_out=partials[:, p : p + 1],
                )
                if mdt is f32:
                    pbs = [pb]
                else:
                    pb_hi = ppool.tile([128, C], mdt, tag="probshi")
                    nc.vector.tensor_copy(pb_hi[:], pb[:])
                    pb_rem = ppool.tile([128, C], f32, tag="probsrem")
                    nc.vector.tensor_sub(pb_rem[:], pb[:], pb_hi[:])
                    pb_lo = ppool.tile([128, C], mdt, tag="probslo")
                    nc.vector.tensor_copy(pb_lo[:], pb_rem[:])
                    pbs = [pb_hi, pb_lo]

                if pending is not None:
                    emit_v_product(*pending)
                pending = (p, kv, pbs)
            emit_v_product(*pending)

            # denominators: den[p] = sum_d partials[d, p] (partials hold exp row-sums)
            den_ps = pden_pool.tile([PAIRS, 1], f32)
            nc.tensor.matmul(den_ps[:], partials[:], ones[:], start=True, stop=True)
            den_sb = small.tile([PAIRS, 1], f32)
            nc.vector.tensor_copy(den_sb[:], den_ps[:])

            nc.sync.dma_start(outT_d[:], outT_sb[:])
            nc.sync.dma_start(den_d[:], den_sb[:])

    nc.compile()
    return nc


def _get_program(variant=None):
    variant = variant or MM_VARIANT
    if variant not in _PROGRAMS:
        _PROGRAMS[variant] = _build_program(variant)
    return _PROGRAMS[variant]


def _split_hi_lo(a, npdt):
    hi = a.astype(npdt)
    lo = (a - hi.astype(np.float32)).astype(npdt)
    return hi, lo


def _prep_core_inputs(q, k, v, mask, core, variant):
    cfg = _cfg(variant)
    npdt = np.float16 if cfg["dt"] is mybir.dt.float16 else np.float32
    h0 = core * HL

    qT = np.ascontiguousarray(
        q[:, h0 : h0 + HL, 0, :].reshape(PAIRS, D).T, dtype=np.float32
    )
    kT = np.ascontiguousarray(
        k[:, h0 : h0 + HL].reshape(PAIRS, S, D).transpose(0, 2, 1), dtype=np.float32
    )
    # vp[p, sp, c, d] = v[p, c*128+sp, d]; flattened to [PAIRS, 128, S]
    vp = np.ascontiguousarray(
        v[:, h0 : h0 + HL].reshape(PAIRS, C, 128, D).transpose(0, 2, 1, 3),
        dtype=np.float32,
    ).reshape(PAIRS, 128, S)

    if npdt is np.float32:
        qT_o = qT.reshape(D, 1, PAIRS)
        slabs = [kT, vp]
    else:
        qh, ql = _split_hi_lo(qT, npdt)
        qT_o = np.stack([qh, ql], axis=1)             # [D, 2, PAIRS]
        if cfg["nk"] == 1:
            slabs = [kT.astype(npdt), vp.astype(npdt)]
        else:
            kh, kl = _split_hi_lo(kT, npdt)
            vh, vl = _split_hi_lo(vp, npdt)
            slabs = [kh, kl, vh, vl]
    kv_o = np.stack(slabs, axis=2).astype(npdt)       # [PAIRS, D, nsl, S]

    maskT = np.ascontiguousarray(
        mask[:, 0, 0, :].reshape(B, C, 128).transpose(2, 0, 1).reshape(128, B * C)
        / SCALE,
        dtype=np.float32,
    )
    return {"qT": qT_o, "kv": kv_o, "maskT": maskT}


def run_sharded(q, k, v, mask, trace=False, variant=None, **kwargs):
    variant = variant or MM_VARIANT
    nc = _get_program(variant)
    in_maps = [_prep_core_inputs(q, k, v, mask, core, variant) for core in range(NCORES)]
    res = run_bass_kernel_spmd(
        nc, in_maps, core_ids=list(range(NCORES)), trace=trace, **kwargs
    )
    out = np.empty((B, H, 1, D), np.float32)
    for core in range(NCORES):
        outT = res.results[core]["outT"]          # [128, 32]
        den = res.results[core]["den"].reshape(PAIRS)
        o = (outT.T / den[:, None]).reshape(B, HL, D)
        out[:, core * HL : (core + 1) * HL, 0, :] = o
    return out, res


def kernel(q, k, v, mask):
    q = np.asarray(q, dtype=np.float32)
    k = np.asarray(k, dtype=np.float32)
    v = np.asarray(v, dtype=np.float32)
    mask = np.asarray(mask, dtype=np.float32)
    out, _ = run_sharded(q, k, v, mask, trace=False)
    return out
